# revision 12
# baseline (speedup 1.0000x reference)
"""Trainium2 Bass kernel for nn_BlockSparseLocallyConnected.

Block-sparse locally-connected layer: 3x3 untied conv on a 32x32 grid,
32->32 channels, batch 128, expressed as 8836 dense 32x32 weight blocks
(BSR). Full inputs in, full output out; internally sharded over 8
NeuronCores by output tile-rows (weights are NOT replicated).

Decomposition: output space is covered by 16x16 spatial tiles of 2x2
positions. For output tile t, contributions come from its 4x4 input
window, which splits into four shifted 2x2 input blocks (passes
(a,b) in {0,1}^2). Each (tile, pass) is ONE tensor-engine matmul
  psum[(v,co), b] += lhsT[(u,ci), (v,co)].T @ rhs[(u,ci), b]
with K = 4 input positions x 32 cin = 128, M = 4 output positions x
32 cout = 128, N = batch = 128, accumulated over the 4 passes in PSUM.

Host pre-packs the input into "row-pair strips" xS[rp, (da,db,ci),
(j,b)] so that every matmul rhs is a contiguous SBUF slice of a strip
(no on-chip data rearrangement at all).

Weights: only the VALID (u, v) slots of each 128x128 lhsT tile travel
over DMA (9 of 16 slots; 2.25 MiB/core instead of 4 MiB). Weight SBUF
tiles are laid out [ (da,db,ci), (v, tj, co) ] so each valid slot is a
contiguous-partition x contiguous-1KiB-run DMA spanning all 16 tj at
once; invalid slots are zeroed once per dedicated buffer. The lhsT AP
for one (tj, pass) is then a 2D strided free AP (v: stride 512, co: 1).

Everything is shipped bf16 (accumulation and output fp32).
"""

import ml_dtypes
import numpy as np

import concourse.bacc as bacc
import concourse.bass as bass
import concourse.mybir as mybir
import concourse.tile as tile
from concourse.bass_utils import run_bass_kernel_spmd

# Problem constants (hardcoded; kernel.py must be self-contained).
B = 128          # batch
C = 32           # channels (in == out)
H = 32           # spatial height == width
NCORES = 8
NTJ = 16         # tile columns (W/2)
NTIL = 2         # tile rows per core (16 tile rows / 8 cores)
NSTRIP = 3       # row-pair strips per core
JSLOTS = 17      # j positions per strip (padded W/2 + 1)
SFREE = JSLOTS * B           # strip free dim = 2176
OCHUNK = 4                   # tj tiles per output DMA chunk (256 KiB)
PASSES = ((0, 0), (0, 1), (1, 0), (1, 1))
F32 = mybir.dt.float32

DT = mybir.dt.bfloat16
NPDT = ml_dtypes.bfloat16

_NC_CACHE = {}


def _wdma_list():
    """Enumerate valid-slot weight DMAs: (til, pass_idx, v, part0, pn).

    Valid da for (a, va): a=0 -> da >= va; a=1 -> da <= va (contiguous
    range either way). Same for db vs (b, vb). One DMA per (til, pass,
    v, da): partitions [da*64 + db0*32, +ndb*32), free block v.
    """
    out = []
    for til in range(NTIL):
        for pi, (a, b) in enumerate(PASSES):
            for v in range(4):
                va, vb = v // 2, v % 2
                das = range(va, 2) if a == 0 else range(0, va + 1)
                dbs = range(vb, 2) if b == 0 else range(0, vb + 1)
                db0, ndb = dbs[0], len(dbs)
                for da in das:
                    out.append((til, pi, v, da * 64 + db0 * 32, ndb * 32))
    return out


WDMAS = _wdma_list()
WROWS = sum(pn for (_, _, _, _, pn) in WDMAS)   # 2304 rows of 512 elems


def _build_nc():
    """Build + compile the SPMD Bass module (one program, 8 cores)."""
    nc = bacc.Bacc(None, target_bir_lowering=False)

    xs_d = nc.dram_tensor("xs", [NSTRIP, 128, SFREE], DT, kind="ExternalInput")
    wt_d = nc.dram_tensor("wt", [WROWS, NTJ, C], DT, kind="ExternalInput")
    bias_d = nc.dram_tensor("bias", [128, NTIL * NTJ], F32, kind="ExternalInput")
    out_d = nc.dram_tensor("out", [NTIL, 128, NTJ * B], F32, kind="ExternalOutput")

    with tile.TileContext(nc) as tc:
        with (
            tc.tile_pool(name="xpool", bufs=NSTRIP) as xpool,
            tc.tile_pool(name="bpool", bufs=1) as bpool,
            tc.tile_pool(name="wpool", bufs=NTIL * 4) as wpool,
            tc.tile_pool(name="opool", bufs=4) as opool,
            tc.tile_pool(name="psum", bufs=8, space="PSUM") as psum,
        ):
            # strips first on the SP ring (compute can't start without them)
            strips = []
            for s in range(NSTRIP):
                st = xpool.tile([128, SFREE], DT, tag="strip")
                nc.sync.dma_start(st[:], xs_d[s])
                strips.append(st)

            bias_t = bpool.tile([128, NTIL * NTJ], F32)
            nc.sync.dma_start(bias_t[:], bias_d[:])

            # dedicated per-(til, pass) weight buffers, [part, v, tj, co];
            # invalid slots stay zero after one memset
            wtiles = {}
            for til in range(NTIL):
                for pi in range(4):
                    wt_t = wpool.tile([128, 4, NTJ, C], DT, tag="w")
                    nc.vector.memset(wt_t[:], 0.0)
                    wtiles[(til, pi)] = wt_t
            row = 0
            for (til, pi, v, p0, pn) in WDMAS:
                nc.scalar.dma_start(
                    wtiles[(til, pi)][p0:p0 + pn, v], wt_d[row:row + pn]
                )
                row += pn

            for til in range(NTIL):
                for tj in range(NTJ):
                    if tj % OCHUNK == 0:
                        out_t = opool.tile([128, OCHUNK * B], F32, tag="out")
                    ps = psum.tile([128, B], F32, tag="acc")
                    for pi, (a, b) in enumerate(PASSES):
                        rhs = strips[til + a][:, (tj + b) * B:(tj + b + 1) * B]
                        for v in range(4):
                            # col-group matmuls run concurrently in the
                            # PE array; lhsT slices stay contiguous
                            lhsT = wtiles[(til, pi)][:, v, tj, :]
                            nc.tensor.matmul(
                                ps[32 * v:32 * (v + 1), :], lhsT, rhs,
                                start=(pi == 0),
                                stop=(pi == 3),
                                tile_position=(0, 32 * v),
                            )
                    # bias add + evacuate PSUM -> SBUF
                    nc.vector.tensor_scalar_add(
                        out_t[:, (tj % OCHUNK) * B:(tj % OCHUNK + 1) * B],
                        ps[:],
                        bias_t[:, til * NTJ + tj:til * NTJ + tj + 1],
                    )
                    if tj % OCHUNK == OCHUNK - 1:
                        o0 = (tj // OCHUNK) * OCHUNK
                        nc.sync.dma_start(
                            out_d[til, :, o0 * B:(o0 + OCHUNK) * B], out_t[:]
                        )

    nc.compile()
    return nc


def _pack_host(input, weight, mask, bias, brow_ids, bcol_ids):
    """Host-side packing of full inputs into per-core device arrays."""
    f32 = np.float32
    x = np.ascontiguousarray(np.asarray(input, dtype=f32))
    vals = np.asarray(weight, dtype=f32) * np.asarray(mask, dtype=f32)
    bias = np.asarray(bias, dtype=f32)
    p_sp = np.asarray(brow_ids).astype(np.int64)
    q_sp = np.asarray(bcol_ids).astype(np.int64)

    # --- input strips: xS[rp, (da,db,ci), (j,b)] = xpad[2rp+da, 2j+db, ci, b]
    x_t = np.transpose(x, (2, 3, 1, 0))                # [h, w, ci, b]
    xpad = np.zeros((H + 2, H + 2, C, B), f32)
    xpad[1:H + 1, 1:H + 1] = x_t
    xS = np.ascontiguousarray(
        xpad.reshape(JSLOTS, 2, JSLOTS, 2, C, B)
        .transpose(0, 1, 3, 4, 2, 5)
        .reshape(JSLOTS, 128, SFREE)
    ).astype(NPDT)

    # --- weights: scatter blocks into padded lhsT tiles, then slice the
    # valid slots into the flat [WROWS, tj, co] DMA stream
    ph, pw = p_sp // H, p_sp % H
    qh, qw = q_sp // H, q_sp % H
    ti, va = ph // 2, ph % 2
    tjc, vb = pw // 2, pw % 2
    ra = qh + 1 - 2 * ti          # = 2a + da in 0..3
    rb = qw + 1 - 2 * tjc         # = 2b + db in 0..3
    aa, da = ra // 2, ra % 2
    bb, db = rb // 2, rb % 2
    core, til = ti // 2, ti % 2
    mm = ((core * NTIL + til) * NTJ + tjc) * 4 + (aa * 2 + bb)
    u = da * 2 + db
    v = va * 2 + vb
    wflat = np.zeros((NCORES * NTIL * NTJ * 4, 4, C, 4, C), NPDT)  # [mm,u,ci,v,co]
    wflat[mm, u, :, v, :] = vals.transpose(0, 2, 1)
    # [c, til, tj, pass, k(u,ci), m(v,co)]
    w6 = wflat.reshape(NCORES, NTIL, NTJ, 4, 128, 128)
    w_cores = []
    for c in range(NCORES):
        rows = []
        for (til, pi, v, p0, pn) in WDMAS:
            # [tj, k-part, co] -> [k-part, tj, co]
            blk = w6[c, til, :, pi, p0:p0 + pn, v * C:(v + 1) * C]
            rows.append(np.ascontiguousarray(blk.transpose(1, 0, 2)))
        w_cores.append(np.concatenate(rows, axis=0))

    # --- bias: [ (va,vb,co), (til,tj) ] per core
    b3 = bias.reshape(H, H, C).reshape(NCORES, NTIL, 2, NTJ, 2, C)
    bias_cores = [
        np.ascontiguousarray(
            b3[c].transpose(1, 3, 4, 0, 2).reshape(128, NTIL * NTJ)
        )
        for c in range(NCORES)
    ]

    in_maps = []
    for c in range(NCORES):
        in_maps.append({
            "xs": np.ascontiguousarray(xS[2 * c:2 * c + NSTRIP]),
            "wt": w_cores[c],
            "bias": bias_cores[c],
        })
    return in_maps


def _unpack_host(results):
    """[c][til, (va,vb,co), (tj,b)] -> [b, co, h, w]"""
    out_all = np.stack([r["out"] for r in results])       # [8, 2, 128, 2048]
    o = out_all.reshape(NCORES, NTIL, 2, 2, C, NTJ, B)    # [c,til,va,vb,co,tj,b]
    o = o.transpose(6, 4, 0, 1, 2, 5, 3)                  # [b,co,c,til,va,tj,vb]
    return np.ascontiguousarray(o.reshape(B, C, H, H))


def kernel(input, weight, mask, bias, brow_ids, bcol_ids, _perf=None):
    if "nc" not in _NC_CACHE:
        _NC_CACHE["nc"] = _build_nc()
    nc = _NC_CACHE["nc"]
    in_maps = _pack_host(input, weight, mask, bias, brow_ids, bcol_ids)
    kwargs = dict(_perf) if _perf else {}
    res = run_bass_kernel_spmd(nc, in_maps, core_ids=list(range(NCORES)), **kwargs)
    if _perf is not None:
        _NC_CACHE["last_result"] = res
    return _unpack_host(res.results)


# revision 13
# speedup vs baseline: 1.5586x; 1.5586x over previous
"""Trainium2 Bass kernel for nn_BlockSparseLocallyConnected.

Block-sparse locally-connected layer: 3x3 untied conv on a 32x32 grid,
32->32 channels, batch 128, expressed as 8836 dense 32x32 weight blocks
(BSR). Full inputs in, full output out; internally sharded over 8
NeuronCores by output tile-rows (weights are NOT replicated).

Decomposition: output space is covered by 16x16 spatial tiles of 2x2
positions. For output tile t, contributions come from its 4x4 input
window, which splits into four shifted 2x2 input blocks (passes
(a,b) in {0,1}^2). Each (tile, pass) is ONE tensor-engine matmul
  psum[(v,co), b] += lhsT[(u,ci), (v,co)].T @ rhs[(u,ci), b]
with K = 4 input positions x 32 cin = 128, M = 4 output positions x
32 cout = 128, N = batch = 128, accumulated over the 4 passes in PSUM.

Host pre-packs the input into "row-pair strips" xS[rp, (da,db,ci),
(j,b)] so that every matmul rhs is a contiguous SBUF slice of a strip
(no on-chip data rearrangement at all).

Weights: only the VALID (u, v) slots of each 128x128 lhsT tile travel
over DMA (9 of 16 slots; 2.25 MiB/core instead of 4 MiB). Weight SBUF
tiles are laid out [ (da,db,ci), (v, tj, co) ] so each valid slot is a
contiguous-partition x contiguous-1KiB-run DMA spanning all 16 tj at
once; invalid slots are zeroed once per dedicated buffer. The lhsT AP
for one (tj, pass) is then a 2D strided free AP (v: stride 512, co: 1).

Everything is shipped bf16 (accumulation and output fp32).
"""

import ml_dtypes
import numpy as np

import concourse.bacc as bacc
import concourse.bass as bass
import concourse.mybir as mybir
import concourse.tile as tile
from concourse.bass_utils import run_bass_kernel_spmd

# Problem constants (hardcoded; kernel.py must be self-contained).
B = 128          # batch
C = 32           # channels (in == out)
H = 32           # spatial height == width
NCORES = 8
NTJ = 16         # tile columns (W/2)
NTIL = 2         # tile rows per core (16 tile rows / 8 cores)
NSTRIP = 3       # row-pair strips per core
JSLOTS = 17      # j positions per strip (padded W/2 + 1)
SFREE = JSLOTS * B           # strip free dim = 2176
OCHUNK = 4                   # tj tiles per output DMA chunk
WCH = 4                      # tj tiles per weight chunk (512 KiB bf16)
PASSES = ((0, 0), (0, 1), (1, 0), (1, 1))
F32 = mybir.dt.float32

DT = mybir.dt.bfloat16
NPDT = ml_dtypes.bfloat16
OUT_BF16 = True
ODT = mybir.dt.bfloat16 if OUT_BF16 else F32
ONP = ml_dtypes.bfloat16 if OUT_BF16 else np.float32

_NC_CACHE = {}


def _build_nc():
    """Build + compile the SPMD Bass module (one program, 8 cores)."""
    nc = bacc.Bacc(None, target_bir_lowering=False)

    xs_d = nc.dram_tensor("xs", [NSTRIP, 128, SFREE], DT, kind="ExternalInput")
    wt_d = nc.dram_tensor("wt", [NTIL, 128, NTJ * 4 * 128], DT, kind="ExternalInput")
    bias_d = nc.dram_tensor("bias", [128, NTIL * NTJ], F32, kind="ExternalInput")
    out_d = nc.dram_tensor("out", [NTIL, 128, NTJ * B], ODT, kind="ExternalOutput")

    with tile.TileContext(nc) as tc:
        with (
            tc.tile_pool(name="xpool", bufs=NSTRIP) as xpool,
            tc.tile_pool(name="bpool", bufs=1) as bpool,
            tc.tile_pool(name="wpool", bufs=NTIL * 4) as wpool,
            tc.tile_pool(name="opool", bufs=4) as opool,
            tc.tile_pool(name="psum", bufs=8, space="PSUM") as psum,
        ):
            # strips first on the SP ring (compute can't start without them)
            strips = []
            for s in range(NSTRIP):
                st = xpool.tile([128, SFREE], DT, tag="strip")
                nc.sync.dma_start(st[:], xs_d[s])
                strips.append(st)

            bias_t = bpool.tile([128, NTIL * NTJ], F32)
            nc.sync.dma_start(bias_t[:], bias_d[:])

            # padded weight chunks on the ACT HWDGE ring; first chunks
            # are small so the PE can start early
            wtiles = {}
            for til in range(NTIL):
                for ci_, tj0 in enumerate(range(0, NTJ, WCH)):
                    wt_t = wpool.tile([128, WCH * 4 * 128], DT, tag="w")
                    nc.scalar.dma_start(
                        wt_t[:],
                        wt_d[til, :, tj0 * 4 * 128:(tj0 + WCH) * 4 * 128],
                    )
                    wtiles[(til, tj0 // WCH)] = wt_t

            for til in range(NTIL):
                for tj in range(NTJ):
                    if tj % OCHUNK == 0:
                        out_t = opool.tile([128, OCHUNK * B], ODT, tag="out")
                    ps = psum.tile([128, B], F32, tag="acc")
                    for pi, (a, b) in enumerate(PASSES):
                        rhs = strips[til + a][:, (tj + b) * B:(tj + b + 1) * B]
                        lhsT = wtiles[(til, tj // WCH)][
                            :, ((tj % WCH) * 4 + pi) * 128:((tj % WCH) * 4 + pi + 1) * 128
                        ]
                        nc.tensor.matmul(
                            ps[:], lhsT, rhs, start=(pi == 0), stop=(pi == 3)
                        )
                    # bias add + evacuate PSUM -> SBUF
                    nc.vector.tensor_scalar_add(
                        out_t[:, (tj % OCHUNK) * B:(tj % OCHUNK + 1) * B],
                        ps[:],
                        bias_t[:, til * NTJ + tj:til * NTJ + tj + 1],
                    )
                    if tj % OCHUNK == OCHUNK - 1:
                        o0 = (tj // OCHUNK) * OCHUNK
                        nc.gpsimd.dma_start(
                            out_d[til, :, o0 * B:(o0 + OCHUNK) * B], out_t[:]
                        )

    nc.compile()
    return nc


def _pack_host(input, weight, mask, bias, brow_ids, bcol_ids):
    """Host-side packing of full inputs into per-core device arrays."""
    f32 = np.float32
    x = np.ascontiguousarray(np.asarray(input, dtype=f32))
    vals = np.asarray(weight, dtype=f32) * np.asarray(mask, dtype=f32)
    bias = np.asarray(bias, dtype=f32)
    p_sp = np.asarray(brow_ids).astype(np.int64)
    q_sp = np.asarray(bcol_ids).astype(np.int64)

    # --- input strips: xS[rp, (da,db,ci), (j,b)] = xpad[2rp+da, 2j+db, ci, b]
    x_t = np.transpose(x, (2, 3, 1, 0))                # [h, w, ci, b]
    xpad = np.zeros((H + 2, H + 2, C, B), f32)
    xpad[1:H + 1, 1:H + 1] = x_t
    xS = np.ascontiguousarray(
        xpad.reshape(JSLOTS, 2, JSLOTS, 2, C, B)
        .transpose(0, 1, 3, 4, 2, 5)
        .reshape(JSLOTS, 128, SFREE)
    ).astype(NPDT)

    # --- weights: scatter blocks into padded lhsT tiles, then slice the
    # valid slots into the flat [WROWS, tj, co] DMA stream
    ph, pw = p_sp // H, p_sp % H
    qh, qw = q_sp // H, q_sp % H
    ti, va = ph // 2, ph % 2
    tjc, vb = pw // 2, pw % 2
    ra = qh + 1 - 2 * ti          # = 2a + da in 0..3
    rb = qw + 1 - 2 * tjc         # = 2b + db in 0..3
    aa, da = ra // 2, ra % 2
    bb, db = rb // 2, rb % 2
    core, til = ti // 2, ti % 2
    mm = ((core * NTIL + til) * NTJ + tjc) * 4 + (aa * 2 + bb)
    u = da * 2 + db
    v = va * 2 + vb
    wflat = np.zeros((NCORES * NTIL * NTJ * 4, 4, C, 4, C), NPDT)  # [mm,u,ci,v,co]
    wflat[mm, u, :, v, :] = vals.transpose(0, 2, 1)
    # -> per-core SBUF layout [til, k=(u,ci), (tj, pass, m=(v,co))]
    w6 = wflat.reshape(NCORES, NTIL, NTJ, 4, 128, 128)
    w_cores = [
        np.ascontiguousarray(
            w6[c].transpose(0, 3, 1, 2, 4).reshape(NTIL, 128, NTJ * 4 * 128)
        )
        for c in range(NCORES)
    ]

    # --- bias: [ (va,vb,co), (til,tj) ] per core
    b3 = bias.reshape(H, H, C).reshape(NCORES, NTIL, 2, NTJ, 2, C)
    bias_cores = [
        np.ascontiguousarray(
            b3[c].transpose(1, 3, 4, 0, 2).reshape(128, NTIL * NTJ)
        )
        for c in range(NCORES)
    ]

    in_maps = []
    for c in range(NCORES):
        in_maps.append({
            "xs": np.ascontiguousarray(xS[2 * c:2 * c + NSTRIP]),
            "wt": w_cores[c],
            "bias": bias_cores[c],
        })
    return in_maps


def _unpack_host(results):
    """[c][til, (va,vb,co), (tj,b)] -> [b, co, h, w]"""
    out_all = np.stack([np.asarray(r["out"], dtype=np.float32) for r in results])
    o = out_all.reshape(NCORES, NTIL, 2, 2, C, NTJ, B)    # [c,til,va,vb,co,tj,b]
    o = o.transpose(6, 4, 0, 1, 2, 5, 3)                  # [b,co,c,til,va,tj,vb]
    return np.ascontiguousarray(o.reshape(B, C, H, H))


def kernel(input, weight, mask, bias, brow_ids, bcol_ids, _perf=None):
    if "nc" not in _NC_CACHE:
        _NC_CACHE["nc"] = _build_nc()
    nc = _NC_CACHE["nc"]
    in_maps = _pack_host(input, weight, mask, bias, brow_ids, bcol_ids)
    kwargs = dict(_perf) if _perf else {}
    res = run_bass_kernel_spmd(nc, in_maps, core_ids=list(range(NCORES)), **kwargs)
    if _perf is not None:
        _NC_CACHE["last_result"] = res
    return _unpack_host(res.results)


# revision 14
# speedup vs baseline: 1.7618x; 1.1303x over previous
"""Trainium2 Bass kernel for nn_BlockSparseLocallyConnected.

Block-sparse locally-connected layer: 3x3 untied conv on a 32x32 grid,
32->32 channels, batch 128, expressed as 8836 dense 32x32 weight blocks
(BSR). Full inputs in, full output out; internally sharded over 8
NeuronCores by output tile-rows (weights are NOT replicated).

Decomposition: output space is covered by 16x16 spatial tiles of 2x2
positions. For output tile t, contributions come from its 4x4 input
window, which splits into four shifted 2x2 input blocks (passes
(a,b) in {0,1}^2). Each (tile, pass) is ONE tensor-engine matmul
  psum[(v,co), b] += lhsT[(u,ci), (v,co)].T @ rhs[(u,ci), b]
with K = 4 input positions x 32 cin = 128, M = 4 output positions x
32 cout = 128, N = batch = 128, accumulated over the 4 passes in PSUM.

Host pre-packs the input into "row-pair strips" xS[rp, (da,db,ci),
(j,b)] so that every matmul rhs is a contiguous SBUF slice of a strip
(no on-chip data rearrangement at all).

Weights: only the VALID (u, v) slots of each 128x128 lhsT tile travel
over DMA (9 of 16 slots; 2.25 MiB/core instead of 4 MiB). Weight SBUF
tiles are laid out [ (da,db,ci), (v, tj, co) ] so each valid slot is a
contiguous-partition x contiguous-1KiB-run DMA spanning all 16 tj at
once; invalid slots are zeroed once per dedicated buffer. The lhsT AP
for one (tj, pass) is then a 2D strided free AP (v: stride 512, co: 1).

Everything is shipped bf16 (accumulation and output fp32).
"""

import ml_dtypes
import numpy as np

import concourse.bacc as bacc
import concourse.bass as bass
import concourse.mybir as mybir
import concourse.tile as tile
from concourse.bass_utils import run_bass_kernel_spmd

# Problem constants (hardcoded; kernel.py must be self-contained).
B = 128          # batch
C = 32           # channels (in == out)
H = 32           # spatial height == width
NCORES = 8
NTJ = 16         # tile columns (W/2)
NTIL = 2         # tile rows per core (16 tile rows / 8 cores)
NSTRIP = 3       # row-pair strips per core
JSLOTS = 17      # j positions per strip (padded W/2 + 1)
SFREE = JSLOTS * B           # strip free dim = 2176
OCHUNK = 8                   # tj tiles per output DMA chunk
WCH = 4                      # tj tiles per weight chunk (512 KiB bf16)
PASSES = ((0, 0), (0, 1), (1, 0), (1, 1))
F32 = mybir.dt.float32

DT = mybir.dt.bfloat16
NPDT = ml_dtypes.bfloat16
OUT_BF16 = True
ODT = mybir.dt.bfloat16 if OUT_BF16 else F32
ONP = ml_dtypes.bfloat16 if OUT_BF16 else np.float32

_NC_CACHE = {}


def _build_nc():
    """Build + compile the SPMD Bass module (one program, 8 cores)."""
    nc = bacc.Bacc(None, target_bir_lowering=False)

    xs_d = nc.dram_tensor("xs", [NSTRIP, 128, SFREE], DT, kind="ExternalInput")
    wt_d = nc.dram_tensor("wt", [NTIL, 128, NTJ * 4 * 128], DT, kind="ExternalInput")
    bias_d = nc.dram_tensor("bias", [128, NTIL * NTJ], F32, kind="ExternalInput")
    out_d = nc.dram_tensor("out", [NTIL, 128, NTJ * B], ODT, kind="ExternalOutput")

    with tile.TileContext(nc) as tc:
        with (
            tc.tile_pool(name="xpool", bufs=NSTRIP) as xpool,
            tc.tile_pool(name="bpool", bufs=1) as bpool,
            tc.tile_pool(name="wpool", bufs=NTIL * 4) as wpool,
            tc.tile_pool(name="opool", bufs=4) as opool,
            tc.tile_pool(name="psum", bufs=8, space="PSUM") as psum,
        ):
            # strips first on the SP ring (compute can't start without them)
            strips = []
            for s in range(NSTRIP):
                st = xpool.tile([128, SFREE], DT, tag="strip")
                nc.sync.dma_start(st[:], xs_d[s])
                strips.append(st)

            bias_t = bpool.tile([128, NTIL * NTJ], F32)
            nc.sync.dma_start(bias_t[:], bias_d[:])

            # padded weight chunks, alternating between the two HWDGE
            # rings so the weight stream gets the full DMA bandwidth
            wtiles = {}
            for i, (til, tj0) in enumerate(
                [(t, j) for t in range(NTIL) for j in range(0, NTJ, WCH)]
            ):
                wt_t = wpool.tile([128, WCH * 4 * 128], DT, tag="w")
                eng = nc.scalar if i % 2 == 0 else nc.sync
                eng.dma_start(
                    wt_t[:],
                    wt_d[til, :, tj0 * 4 * 128:(tj0 + WCH) * 4 * 128],
                )
                wtiles[(til, tj0 // WCH)] = wt_t

            for til in range(NTIL):
                for tj in range(NTJ):
                    if tj % OCHUNK == 0:
                        out_t = opool.tile([128, OCHUNK * B], ODT, tag="out")
                    ps = psum.tile([128, B], F32, tag="acc")
                    for pi, (a, b) in enumerate(PASSES):
                        rhs = strips[til + a][:, (tj + b) * B:(tj + b + 1) * B]
                        lhsT = wtiles[(til, tj // WCH)][
                            :, ((tj % WCH) * 4 + pi) * 128:((tj % WCH) * 4 + pi + 1) * 128
                        ]
                        nc.tensor.matmul(
                            ps[:], lhsT, rhs, start=(pi == 0), stop=(pi == 3)
                        )
                    # bias add + evacuate PSUM -> SBUF
                    nc.vector.tensor_scalar_add(
                        out_t[:, (tj % OCHUNK) * B:(tj % OCHUNK + 1) * B],
                        ps[:],
                        bias_t[:, til * NTJ + tj:til * NTJ + tj + 1],
                    )
                    if tj % OCHUNK == OCHUNK - 1:
                        o0 = (tj // OCHUNK) * OCHUNK
                        nc.gpsimd.dma_start(
                            out_d[til, :, o0 * B:(o0 + OCHUNK) * B], out_t[:]
                        )

    nc.compile()
    return nc


def _pack_host(input, weight, mask, bias, brow_ids, bcol_ids):
    """Host-side packing of full inputs into per-core device arrays."""
    f32 = np.float32
    x = np.ascontiguousarray(np.asarray(input, dtype=f32))
    vals = np.asarray(weight, dtype=f32) * np.asarray(mask, dtype=f32)
    bias = np.asarray(bias, dtype=f32)
    p_sp = np.asarray(brow_ids).astype(np.int64)
    q_sp = np.asarray(bcol_ids).astype(np.int64)

    # --- input strips: xS[rp, (da,db,ci), (j,b)] = xpad[2rp+da, 2j+db, ci, b]
    x_t = np.transpose(x, (2, 3, 1, 0))                # [h, w, ci, b]
    xpad = np.zeros((H + 2, H + 2, C, B), f32)
    xpad[1:H + 1, 1:H + 1] = x_t
    xS = np.ascontiguousarray(
        xpad.reshape(JSLOTS, 2, JSLOTS, 2, C, B)
        .transpose(0, 1, 3, 4, 2, 5)
        .reshape(JSLOTS, 128, SFREE)
    ).astype(NPDT)

    # --- weights: scatter blocks into padded lhsT tiles, then slice the
    # valid slots into the flat [WROWS, tj, co] DMA stream
    ph, pw = p_sp // H, p_sp % H
    qh, qw = q_sp // H, q_sp % H
    ti, va = ph // 2, ph % 2
    tjc, vb = pw // 2, pw % 2
    ra = qh + 1 - 2 * ti          # = 2a + da in 0..3
    rb = qw + 1 - 2 * tjc         # = 2b + db in 0..3
    aa, da = ra // 2, ra % 2
    bb, db = rb // 2, rb % 2
    core, til = ti // 2, ti % 2
    mm = ((core * NTIL + til) * NTJ + tjc) * 4 + (aa * 2 + bb)
    u = da * 2 + db
    v = va * 2 + vb
    wflat = np.zeros((NCORES * NTIL * NTJ * 4, 4, C, 4, C), NPDT)  # [mm,u,ci,v,co]
    wflat[mm, u, :, v, :] = vals.transpose(0, 2, 1)
    # -> per-core SBUF layout [til, k=(u,ci), (tj, pass, m=(v,co))]
    w6 = wflat.reshape(NCORES, NTIL, NTJ, 4, 128, 128)
    w_cores = [
        np.ascontiguousarray(
            w6[c].transpose(0, 3, 1, 2, 4).reshape(NTIL, 128, NTJ * 4 * 128)
        )
        for c in range(NCORES)
    ]

    # --- bias: [ (va,vb,co), (til,tj) ] per core
    b3 = bias.reshape(H, H, C).reshape(NCORES, NTIL, 2, NTJ, 2, C)
    bias_cores = [
        np.ascontiguousarray(
            b3[c].transpose(1, 3, 4, 0, 2).reshape(128, NTIL * NTJ)
        )
        for c in range(NCORES)
    ]

    in_maps = []
    for c in range(NCORES):
        in_maps.append({
            "xs": np.ascontiguousarray(xS[2 * c:2 * c + NSTRIP]),
            "wt": w_cores[c],
            "bias": bias_cores[c],
        })
    return in_maps


def _unpack_host(results):
    """[c][til, (va,vb,co), (tj,b)] -> [b, co, h, w]"""
    out_all = np.stack([np.asarray(r["out"], dtype=np.float32) for r in results])
    o = out_all.reshape(NCORES, NTIL, 2, 2, C, NTJ, B)    # [c,til,va,vb,co,tj,b]
    o = o.transpose(6, 4, 0, 1, 2, 5, 3)                  # [b,co,c,til,va,tj,vb]
    return np.ascontiguousarray(o.reshape(B, C, H, H))


def kernel(input, weight, mask, bias, brow_ids, bcol_ids, _perf=None):
    if "nc" not in _NC_CACHE:
        _NC_CACHE["nc"] = _build_nc()
    nc = _NC_CACHE["nc"]
    in_maps = _pack_host(input, weight, mask, bias, brow_ids, bcol_ids)
    kwargs = dict(_perf) if _perf else {}
    res = run_bass_kernel_spmd(nc, in_maps, core_ids=list(range(NCORES)), **kwargs)
    if _perf is not None:
        _NC_CACHE["last_result"] = res
    return _unpack_host(res.results)
